# revision 11
# baseline (speedup 1.0000x reference)
"""Trainium2 Bass kernel for nn_Attention_45578192945380 — pipelined v2.

Sharding: core c -> batch b=c//2, head group g=c%2 (4 heads = 2 cc chunks).
Partial out-projections (bf16) summed on host.

Structure (vs the phase-separated v1: ramp 62us + exp 139us + tail 31us):
  - x arrives fp16; LayerNorm folds into the PE transpose: DVE centers x
    (x - mu, per-partition scalar), PE multiplies by diag(rstd) while
    transposing (fp16 matmul). No separate LN-apply pass.
  - rstd comes from a DVE-only Newton rsqrt (seed 1.5 - a/2 clamped, 4
    iters): the ACT engine never leaves the exp table set (each walrus
    table switch costs ~1.3us and Ln/Exp live in different sets here).
  - attention emission interleaves with phase-1 groups: the exp stream
    starts once groups 0-1 are projected; groups 2-3 + the cc=1 qk pieces
    ride the per-slot PE slack of later units.
  - during the ramp the (otherwise idle) ACT engine does all PSUM
    evacuations so the DVE queue never gates the PE via the dps rotation.
  - per J all dots share one kT weight load; v padded to 128 columns for
    fast weight load; softmax denominator = ones column of v (av row 64);
    unnormalized softmax is safe (|scores| <~ 10).
  - denominator broadcast via gpsimd partition_broadcast (needs a
    partition-0 staging row; it replicates the physical partition 0).
  - h3 runs i-halves sequentially: out-proj + stores of ih0 overlap the
    ih1 exp stream; the final boundary reads the PSUM accumulator
    directly to shorten the serial tail.
  - single shared PSUM pool: dps tag 2x[128,1024] + av tag 2x[128,1024]
    = 8 banks; transposes/QKV/out-proj carve regions of dps-tag tiles.
  - identity matrix, fp16 x, bf16 weights and weight layouts prepped on
    host; x t0-7 on the sync queue, t8-15 + weights batched on gpsimd.
"""

import os
import sys
from contextlib import ExitStack

import numpy as np

for _p in ("/opt/trn_rl_repo", "/root/.axon_site/_ro/trn_rl_repo"):
    if os.path.isdir(_p) and _p not in sys.path:
        sys.path.insert(0, _p)

import ml_dtypes

import concourse.bass as bass
import concourse.bacc as bacc
import concourse.tile as tile
from concourse import mybir
from concourse.bass_utils import run_bass_kernel_spmd

F32 = mybir.dt.float32
F16 = mybir.dt.float16
BF16 = mybir.dt.bfloat16
AF = mybir.ActivationFunctionType
OP = mybir.AluOpType
BFNP = ml_dtypes.bfloat16

B, N, DIM = 4, 2048, 512
HEADS, DH = 8, 64
EPS = 1e-5
NT = N // 128           # 16 n-tiles
DC = DIM // 128         # 4 d-chunks
SCALE = DH ** -0.5
NCORES = 8


def _emit(tc: tile.TileContext, ctx: ExitStack, aps: dict, affine: bool):
    nc = tc.nc

    const = ctx.enter_context(tc.tile_pool(name="const", bufs=1))
    big = ctx.enter_context(tc.tile_pool(name="big", bufs=1))

    # ---- weights / constants (gpsimd HWDGE queue; sync queue owns x) ----
    wq_sb = const.tile([128, DC, 256], BF16)
    wk_sb = const.tile([128, DC, 256], BF16)
    wv_sb = const.tile([128, DC, 256], BF16)
    wo_sb = const.tile([128, 2, 512], BF16)
    # one batched DMA per weight tensor: each gpsimd dma_start costs ~630ns
    # of engine time, so 14 separate triggers would stall the queue ~9us
    nc.gpsimd.dma_start(out=wq_sb[:, :, :],
                        in_=aps["wq"].rearrange("(dc p) c -> p dc c", p=128))
    nc.gpsimd.dma_start(out=wk_sb[:, :, :],
                        in_=aps["wk"].rearrange("(dc p) c -> p dc c", p=128))
    nc.gpsimd.dma_start(out=wv_sb[:, :, :],
                        in_=aps["wv"].rearrange("(dc p) c -> p dc c", p=128))
    nc.gpsimd.dma_start(out=wo_sb[:, :, :],
                        in_=aps["wo"].rearrange("(cc p) c -> p cc c", p=128))
    cadd_sb = None
    ones_row = None
    if affine:
        cadd_sb = const.tile([1, 768], BF16)     # beta @ w rows: q|k|v
        nc.gpsimd.dma_start(out=cadd_sb[:, :], in_=aps["cadd"][:, :])
        ones_row = const.tile([1, N], BF16)
        nc.vector.memset(ones_row, 1.0)
    pb_sb = const.tile([128, NT], F32)
    nc.gpsimd.dma_start(out=pb_sb[:, :], in_=aps["pb"].rearrange("(t p) -> p t", p=128))

    # ---- persistent activations ----
    x_sb = big.tile([128, NT, DIM], F16)         # 16 KB/part
    xT = big.tile([128, DC, N], BF16)            # 16 KB/part
    qT = big.tile([128, 2, N], BF16)             # 8 KB/part
    kT = big.tile([128, 2, N], BF16)
    v_sb = big.tile([128, NT, 4, 128], BF16)     # 16 KB/part (64 v | ones | 0pad)
    aoT = big.tile([128, 2, N], BF16)
    stats = const.tile([128, NT, 2], F32)
    xsum = const.tile([128, 8], F32)
    xsq = const.tile([128, 8], F32)
    lnv = const.tile([128, NT], F32)
    rstd = const.tile([128, NT], F32)
    negmu = const.tile([128, NT], F32)
    eps_sb = const.tile([128, 1], F32)
    nc.vector.memset(eps_sb, EPS)
    zero_sb = const.tile([128, 1], F32)
    nc.vector.memset(zero_sb, 0.0)
    ident16 = const.tile([128, 128], F16)
    # zero v_sb on the ACT engine (idle until the first exp ~20us); the ones
    # column on DVE (tiny). Keeps both the DVE queue and gpsimd queue clear.
    # only the pad columns need zeroing (0-63 are written by the v evacs,
    # col 64 becomes the ones column right after)
    nc.scalar.memzero(v_sb[:, :, :, 64:128])
    nc.vector.memset(v_sb[:, :, :, 64:65], 1.0)

    # all x loads issued up-front, split across four HWDGE queues
    # x t0-7 on sync (feeds groups 0-1); t8-15 on gpsimd behind the weights
    for t in range(8):
        nc.sync.dma_start(out=x_sb[:, t, :], in_=aps["x"][t * 128:(t + 1) * 128, :])
        if t == 3:
            nc.sync.dma_start(out=ident16[:, :], in_=aps["ident"][:, :])
    for t in range(8, NT):
        nc.gpsimd.dma_start(out=x_sb[:, t, :], in_=aps["x"][t * 128:(t + 1) * 128, :])

    # ---- pools (whole-kernel scope; PSUM: 2x2 + 2x2 = 8 banks) ----
    ps_pool = ctx.enter_context(tc.tile_pool(name="ps", bufs=2, space="PSUM"))
    av_pool = ctx.enter_context(tc.tile_pool(name="avps", bufs=2, space="PSUM"))
    epool = ctx.enter_context(tc.tile_pool(name="epool", bufs=8))
    ph1 = ctx.enter_context(tc.tile_pool(name="ph1", bufs=3))
    avsb_pool = ctx.enter_context(tc.tile_pool(name="avsb", bufs=2))
    rbc_pool = ctx.enter_context(tc.tile_pool(name="rbc", bufs=2))
    ostage = ctx.enter_context(tc.tile_pool(name="ostage", bufs=4))
    dram_pool = ctx.enter_context(tc.tile_pool(name="dramb", bufs=2, space="DRAM"))

    def dps_tile(name):
        return ps_pool.tile([128, 1024], F32, tag="dps", name=name)

    # ---------------- phase-1 building blocks ----------------
    def bn_tile(t):
        st6 = ph1.tile([128, 6], F32, tag="bnst")
        nc.vector.bn_stats(out=st6, in_=x_sb[:, t, :])
        nc.vector.bn_aggr(out=stats[:, t, :], in_=st6)

    def act_stats_tile(t):
        # ramp tiles: sum and sum-of-squares ride the (idle) ACT engine's
        # accum_out reduction, freeing the DVE queue which gates the ramp
        dum = ph1.tile([128, DIM], F16, tag="dum")
        nc.scalar.activation(out=dum, in_=x_sb[:, t, :], func=AF.Identity,
                             accum_out=xsum[:, t:t + 1])
        dum2 = ph1.tile([128, DIM], F16, tag="dum")
        nc.scalar.activation(out=dum2, in_=x_sb[:, t, :], func=AF.Square,
                             accum_out=xsq[:, t:t + 1])

    def moments_from_act(sl, w):
        # negmu = -sum/512 ; a = sumsq/512 + eps - mu^2  (tiny DVE ops)
        nc.vector.tensor_scalar(out=negmu[:, sl], in0=xsum[:, sl],
                                scalar1=-1.0 / DIM, scalar2=None, op0=OP.mult)
        msq = ph1.tile([128, w], F32, tag="nwt")
        nc.vector.tensor_tensor(out=msq, in0=negmu[:, sl], in1=negmu[:, sl], op=OP.mult)
        a = lnv[:, sl]
        nc.vector.tensor_scalar(out=a, in0=xsq[:, sl], scalar1=1.0 / DIM,
                                scalar2=EPS, op0=OP.mult, op1=OP.add)
        nc.vector.tensor_tensor(out=a, in0=a, in1=msq, op=OP.subtract)

    def rsqrt_slice(sl, w, from_act=False):
        # rstd via DVE-only Newton rsqrt: the ACT engine never leaves the exp
        # table set (a table load costs 1.3us and walrus reloads per switch).
        # seed y0 = clamp(1.5 - a/2, >=0.2): exact at a=1 (LN variance ~1)
        a = lnv[:, sl]
        if from_act:
            moments_from_act(sl, w)
        else:
            nc.vector.tensor_scalar(out=a, in0=stats[:, sl, 1],
                                    scalar1=EPS, scalar2=None, op0=OP.add)
        y = rstd[:, sl]
        nc.vector.tensor_scalar(out=y, in0=a, scalar1=-0.5, scalar2=1.5,
                                op0=OP.mult, op1=OP.add)
        nc.vector.tensor_scalar(out=y, in0=y, scalar1=0.2, scalar2=None,
                                op0=OP.max)
        for _ in range(4):
            t2 = ph1.tile([128, w], F32, tag="nwt")
            nc.vector.tensor_tensor(out=t2, in0=y, in1=y, op=OP.mult)
            nc.vector.tensor_tensor(out=t2, in0=t2, in1=a, op=OP.mult)
            nc.vector.tensor_scalar(out=t2, in0=t2, scalar1=-0.5, scalar2=1.5,
                                    op0=OP.mult, op1=OP.add)
            nc.vector.tensor_tensor(out=y, in0=y, in1=t2, op=OP.mult)
        if not from_act:
            nc.vector.tensor_scalar(out=negmu[:, sl], in0=stats[:, sl, 0],
                                    scalar1=-1.0, scalar2=None, op0=OP.mult)

    def stats_group(g):
        for t in range(4 * g, 4 * g + 4):
            bn_tile(t)
        rsqrt_slice(slice(4 * g, 4 * g + 4), 4)

    def transpose_tile(t, ramp=False):
        # xc = x - mu (DVE, per-partition scalar); xT = xc.T @ diag(rstd) on PE
        xc = ph1.tile([128, DIM], F16, tag="xc")
        nc.vector.tensor_scalar(out=xc, in0=x_sb[:, t, :],
                                scalar1=negmu[:, t:t + 1], scalar2=None, op0=OP.add)
        diag = ph1.tile([128, 128], F16, tag="diag")
        nc.vector.tensor_scalar(out=diag, in0=ident16, scalar1=rstd[:, t:t + 1],
                                scalar2=None, op0=OP.mult)
        ps = dps_tile(f"tp_{t}")
        for dc in range(DC):
            nc.tensor.matmul(ps[:, dc * 128:(dc + 1) * 128],
                             xc[:, dc * 128:(dc + 1) * 128], diag,
                             start=True, stop=True)
        # pre-stream the ACT engine is idle: let it do the evacuations so the
        # DVE queue never gates the PE through the dps rotation
        eng = nc.scalar.copy if ramp else (lambda out, in_: nc.vector.tensor_copy(out=out, in_=in_))
        eng(out=xT[:, :, t * 128:(t + 1) * 128],
            in_=ps[:, 0:512].rearrange("p (dc c) -> p dc c", dc=4))

    def qk_piece(w_sb, woff, dst, cc, p, ramp=False):
        ps = dps_tile(f"qk_{woff}_{cc}_{p}")
        if affine:
            nc.tensor.matmul(ps[:, 0:512], cadd_sb[0:1, woff + cc * 128: woff + (cc + 1) * 128],
                             ones_row[0:1, p * 512:(p + 1) * 512], start=True, stop=False)
        for dc in range(DC):
            nc.tensor.matmul(ps[:, 0:512], w_sb[:, dc, cc * 128:(cc + 1) * 128],
                             xT[:, dc, p * 512:(p + 1) * 512],
                             start=(dc == 0 and not affine), stop=(dc == DC - 1))
        if ramp:
            nc.scalar.copy(out=dst[:, cc, p * 512:(p + 1) * 512], in_=ps[:, 0:512])
        else:
            nc.vector.tensor_copy(out=dst[:, cc, p * 512:(p + 1) * 512], in_=ps[:, 0:512])

    def v_chunk(t, ramp=False):
        ps = dps_tile(f"v_{t}")
        if affine:
            nc.tensor.matmul(ps[:, 0:256], ones_row[0:1, t * 128:(t + 1) * 128],
                             cadd_sb[0:1, 512:768], start=True, stop=False)
        for dc in range(DC):
            nc.tensor.matmul(ps[:, 0:256], xT[:, dc, t * 128:(t + 1) * 128],
                             wv_sb[:, dc, :], start=(dc == 0 and not affine),
                             stop=(dc == DC - 1))
        # one strided copy: [128, 4, 64] view of v_sb (ACT pre-stream)
        if ramp:
            nc.scalar.copy(out=v_sb[:, t, :, 0:64],
                           in_=ps[:, 0:256].rearrange("p (h d) -> p h d", h=4))
        else:
            nc.vector.tensor_copy(out=v_sb[:, t, :, 0:64],
                                  in_=ps[:, 0:256].rearrange("p (h d) -> p h d", h=4))

    def phase1_group(g, ccs, defer_v=False, ramp=False):
        stats_group(g)
        for t in range(4 * g, 4 * g + 4):
            transpose_tile(t, ramp=ramp)
        for cc in ccs:
            qk_piece(wq_sb, 0, qT, cc, g, ramp=ramp)
            qk_piece(wk_sb, 256, kT, cc, g, ramp=ramp)
        if not defer_v:
            for t in range(4 * g, 4 * g + 4):
                v_chunk(t, ramp=ramp)

    # ---------------- attention building blocks ----------------
    def emit_dots_exp(h, ih, J):
        cc, off = h // 2, 64 * (h % 2)
        ps = dps_tile(f"d_{h}_{ih}_{J}")
        for p in range(2):
            nc.tensor.matmul(
                ps[:, p * 512:(p + 1) * 512],
                kT[off:off + 64, cc, J * 128:(J + 1) * 128],
                qT[off:off + 64, cc, ih * 1024 + p * 512: ih * 1024 + (p + 1) * 512],
                start=True, stop=True)
        eT = epool.tile([128, 1024], BF16, tag="e")
        nc.scalar.activation(out=eT, in_=ps, func=AF.Exp,
                             bias=pb_sb[:, J:J + 1], scale=SCALE)
        return eT

    def emit_av(h, av_ps, J, eT):
        for p in range(2):
            nc.tensor.matmul(av_ps[:, p * 512:(p + 1) * 512],
                             v_sb[:, J, h, :], eT[:, p * 512:(p + 1) * 512],
                             start=(J == 0), stop=(J == NT - 1))

    def boundary(h, ih, av_ps, use_act=False, final=False):
        # av -> aoT (normalized); denominator row 64 -> bcast -> reciprocal
        cc, off = h // 2, 64 * (h % 2)
        if final:
            # last unit: read the PSUM accumulator directly in the normalize
            # multiply (shortens the serial tail after the last exp)
            av_sb = av_ps
            den_row = rbc_pool.tile([1, 1024], F32, tag="drow")
            nc.vector.tensor_copy(out=den_row, in_=av_ps[64:65, :])
        else:
            av_sb = avsb_pool.tile([65, 1024], F32, tag="avsb")
            if use_act:
                nc.scalar.copy(out=av_sb, in_=av_ps[0:65, :])
            else:
                nc.vector.tensor_copy(out=av_sb, in_=av_ps[0:65, :])
            den_row = rbc_pool.tile([1, 1024], F32, tag="drow")
            nc.vector.tensor_copy(out=den_row, in_=av_sb[64:65, :])
        d_bc = rbc_pool.tile([64, 1024], F32, tag="dbc")
        # partition_broadcast replicates the tile's physical partition 0, so
        # the denominator must be copied to a partition-0 tile first
        nc.gpsimd.partition_broadcast(d_bc, den_row[:, :])
        r_bc = rbc_pool.tile([64, 1024], F32, tag="rbc")
        nc.vector.reciprocal_approx_fast(out=r_bc, in_=d_bc)
        nc.vector.tensor_tensor(out=aoT[off:off + 64, cc, ih * 1024:(ih + 1) * 1024],
                                in0=av_sb[0:64, :], in1=r_bc, op=OP.mult)

    def outproj_piece(p, mc, tail=False):
        po = dps_tile(f"op_{p}_{mc}")
        for ccx in range(2):
            nc.tensor.matmul(po[:, 0:512], wo_sb[:, ccx, mc * 128:(mc + 1) * 128],
                             aoT[:, ccx, p * 512:(p + 1) * 512],
                             start=(ccx == 0), stop=(ccx == 1))
        st = ostage.tile([128, 512], BF16, tag="ost")
        if tail and mc % 2 == 0:
            nc.scalar.copy(out=st, in_=po[:, 0:512])
        else:
            nc.vector.tensor_copy(out=st, in_=po[:, 0:512])
        oeng = nc.sync if mc % 2 == 0 else nc.scalar
        oeng.dma_start(out=aps["out"][mc * 128:(mc + 1) * 128, p * 512:(p + 1) * 512],
                       in_=st)

    # sequential-unit emitter with a 2-deep AV software pipeline
    class Unit:
        def __init__(self, h, ih):
            self.h, self.ih = h, ih
            self.av = av_pool.tile([128, 1024], F32, tag="av", name=f"av_{h}_{ih}")
            self.pend = []

        def run(self, Js, extra=None):
            for idx, J in enumerate(Js):
                eT = emit_dots_exp(self.h, self.ih, J)
                if len(self.pend) >= 2:
                    Jp, eTp = self.pend.pop(0)
                    emit_av(self.h, self.av, Jp, eTp)
                self.pend.append((J, eT))
                if extra is not None:
                    extra(idx)

        def finish(self, use_act=False, final=False):
            for Jp, eTp in self.pend:
                emit_av(self.h, self.av, Jp, eTp)
            self.pend = []
            boundary(self.h, self.ih, self.av, use_act=use_act, final=final)

    # ---------------- schedule ----------------
    # PE warmup: dummy matmuls as soon as the identity lands (~7us), then
    # more keyed on each landing x tile so the PE stays busy (HAM warm)
    # through the stats window until the real transposes start.
    wu = dps_tile("warmup")
    for i in range(16):
        nc.tensor.matmul(wu[:, (i % 4) * 128:(i % 4) * 128 + 128],
                         ident16, ident16, start=True, stop=True)
    phase1_group(0, ccs=[0], ramp=True)
    phase1_group(1, ccs=[0], ramp=True)
    phase1_group(1, ccs=[0], ramp=True)

    u = Unit(0, 0)
    u.run(range(0, 8))
    phase1_group(2, ccs=[0])
    qk_piece(wq_sb, 0, qT, 1, 0)
    qk_piece(wk_sb, 256, kT, 1, 0)
    u.run(range(8, 12))
    phase1_group(3, ccs=[0], defer_v=True)
    qk_piece(wq_sb, 0, qT, 1, 1)

    def u3_extra(idx):
        if idx < 4:
            v_chunk(12 + idx)

    u.run(range(12, 16), extra=u3_extra)
    u.finish()

    u = Unit(0, 1)

    def u01_extra(idx):
        if idx == 4:
            qk_piece(wk_sb, 256, kT, 1, 1)
        elif idx == 10:
            qk_piece(wq_sb, 0, qT, 1, 2)

    u.run(range(0, 16), extra=u01_extra)
    u.finish()

    # remaining cc=1 qk pieces ride the small per-J slack in h1
    h1_pieces = [(wk_sb, 256, kT, 2), (wq_sb, 0, qT, 3), (wk_sb, 256, kT, 3)]

    def h1_extra(idx):
        if idx % 5 == 2 and idx // 5 < len(h1_pieces):
            w, woff, dst, g = h1_pieces[idx // 5]
            qk_piece(w, woff, dst, 1, g)

    # h1, h2: interleaved i-halves (4 dots share one kT load per J)
    for h in (1, 2):
        u0, u1 = Unit(h, 0), Unit(h, 1)
        pend = []
        for J in range(NT):
            e0 = emit_dots_exp(h, 0, J)
            e1 = emit_dots_exp(h, 1, J)
            if len(pend) >= 2:
                Jp, p0, p1 = pend.pop(0)
                emit_av(h, u0.av, Jp, p0)
                emit_av(h, u1.av, Jp, p1)
            pend.append((J, e0, e1))
            if h == 1:
                h1_extra(J)
        for Jp, p0, p1 in pend:
            emit_av(h, u0.av, Jp, p0)
            emit_av(h, u1.av, Jp, p1)
        boundary(h, 0, u0.av)
        boundary(h, 1, u1.av)

    # h3: sequential i-halves; out-proj(ih0) overlaps ih1's exp stream
    u = Unit(3, 0)
    u.run(range(0, 16))
    u.finish()

    u = Unit(3, 1)
    op_state = {"i": 0}

    def h3_extra(idx):
        # start emitting ih0 out-proj pieces once the normalize chain has had
        # time to land (after a few J); one piece every other J
        if idx >= 6 and idx % 2 == 0 and op_state["i"] < 8:
            i = op_state["i"]
            outproj_piece(i // 4, i % 4)
            op_state["i"] += 1

    u.run(range(0, 16), extra=h3_extra)
    u.finish(use_act=True, final=True)
    wu2 = dps_tile("warmup2")
    for i in range(24):
        nc.tensor.matmul(wu2[:, (i % 4) * 128:(i % 4) * 128 + 128],
                         ident16, ident16, start=True, stop=True)
    while op_state["i"] < 8:
        i = op_state["i"]
        outproj_piece(i // 4, i % 4)
        op_state["i"] += 1
    for i in range(8):
        outproj_piece(2 + i // 4, i % 4, tail=True)

    if "d_xT" in aps:  # debug dumps
        nc.sync.dma_start(out=aps["d_rstd"][:, :], in_=rstd[:, :])
        for dc in range(DC):
            nc.sync.dma_start(out=aps["d_xT"][:, dc, :], in_=xT[:, dc, :])
        for cc in range(2):
            nc.sync.dma_start(out=aps["d_qT"][:, cc, :], in_=qT[:, cc, :])
            nc.sync.dma_start(out=aps["d_kT"][:, cc, :], in_=kT[:, cc, :])
            nc.sync.dma_start(out=aps["d_aoT"][:, cc, :], in_=aoT[:, cc, :])
        for t in range(NT):
            nc.sync.dma_start(out=aps["d_v"][:, t, :, :], in_=v_sb[:, t, :, :])


_CACHE: dict = {}


def _build(affine: bool):
    key = ("nc", affine)
    if key in _CACHE:
        return _CACHE[key]
    nc = bacc.Bacc("TRN2", target_bir_lowering=False, debug=False,
                   num_devices=NCORES)
    aps = {
        "x": nc.dram_tensor("x", [N, DIM], F16, kind="ExternalInput").ap(),
        "pb": nc.dram_tensor("pb", [N], F32, kind="ExternalInput").ap(),
        "wq": nc.dram_tensor("wq", [DIM, 256], BF16, kind="ExternalInput").ap(),
        "wk": nc.dram_tensor("wk", [DIM, 256], BF16, kind="ExternalInput").ap(),
        "wv": nc.dram_tensor("wv", [DIM, 256], BF16, kind="ExternalInput").ap(),
        "wo": nc.dram_tensor("wo", [256, DIM], BF16, kind="ExternalInput").ap(),
        "ident": nc.dram_tensor("ident", [128, 128], F16, kind="ExternalInput").ap(),
        "out": nc.dram_tensor("out", [DIM, N], BF16, kind="ExternalOutput").ap(),
    }
    if affine:
        aps["cadd"] = nc.dram_tensor("cadd", [1, 768], BF16, kind="ExternalInput").ap()
    with tile.TileContext(nc) as tc:
        with ExitStack() as ctx:
            _emit(tc, ctx, aps, affine)
    nc.compile()
    _CACHE[key] = nc
    return nc


def _prep_in_maps(x, pose_bias, ln_gamma, ln_beta, w_qkv, w_out, beta):
    x = np.asarray(x, np.float32)
    pose = np.asarray(pose_bias, np.float32)
    gam = np.asarray(ln_gamma, np.float32)
    bet = np.asarray(ln_beta, np.float32)
    wqkv = np.asarray(w_qkv, np.float32)
    wo = np.asarray(w_out, np.float32)
    bval = float(np.asarray(beta))
    affine = not (np.all(gam == 1.0) and np.all(bet == 0.0))
    wqkv_eff = wqkv * gam[:, None] if affine else wqkv
    wq_b = wqkv_eff[:, 0:512].astype(BFNP)
    wk_b = wqkv_eff[:, 512:1024].astype(BFNP)
    wv_b = wqkv_eff[:, 1024:1536].astype(BFNP)
    if affine:
        cadd = bet @ wqkv  # [1536] f32
    in_maps = []
    for c in range(NCORES):
        b, g = c // 2, c % 2
        sl = slice(g * 256, (g + 1) * 256)
        m = {
            "x": np.ascontiguousarray(x[b]).astype(np.float16),
            "pb": np.ascontiguousarray(bval * pose[b]),
            "wq": np.ascontiguousarray(wq_b[:, sl]),
            "wk": np.ascontiguousarray(wk_b[:, sl]),
            "wv": np.ascontiguousarray(wv_b[:, sl]),
            "wo": np.ascontiguousarray(wo[sl, :]).astype(BFNP),
            "ident": np.eye(128, dtype=np.float16),
        }
        if affine:
            m["cadd"] = np.ascontiguousarray(
                np.concatenate([cadd[0:512][sl], cadd[512:1024][sl],
                                cadd[1024:1536][sl]])[None, :].astype(BFNP))
        in_maps.append(m)
    return in_maps, affine


def _gather(results):
    outs = []
    for b in range(B):
        o = results[2 * b]["out"].astype(np.float32) + results[2 * b + 1]["out"].astype(np.float32)
        outs.append(o.T)
    return np.ascontiguousarray(np.stack(outs))


def _ensure_ntff_shim():
    import types
    if "antenv.axon_hooks" in sys.modules:
        return
    mod = types.ModuleType("antenv.axon_hooks")
    state = {"hook": None}
    mod.set_axon_ntff_profile_hook = lambda h: state.__setitem__("hook", h)
    mod.get_axon_ntff_profile_hook = lambda: state["hook"]
    sys.modules["antenv.axon_hooks"] = mod
    try:
        from trn_agent_boot.trn_boot import _ntff_profile_via_ctypes
        mod.set_axon_ntff_profile_hook(
            _ntff_profile_via_ctypes("/opt/axon/libaxon_pjrt.so"))
    except Exception:
        pass


def run(trace=False, **inputs):
    if trace:
        _ensure_ntff_shim()
    in_maps, affine = _prep_in_maps(**inputs)
    nc = _build(affine)
    res = run_bass_kernel_spmd(nc, in_maps, core_ids=list(range(NCORES)),
                               trace=trace)
    return _gather(res.results), res


def kernel(**inputs) -> np.ndarray:
    out, _ = run(trace=False, **inputs)
    return out


# revision 12
# speedup vs baseline: 1.1772x; 1.1772x over previous
"""Trainium2 Bass kernel for nn_Attention_45578192945380 — pipelined v2.

Sharding: core c -> batch b=c//2, head group g=c%2 (4 heads = 2 cc chunks).
Partial out-projections (bf16) summed on host.

Structure (vs the phase-separated v1: ramp 62us + exp 139us + tail 31us):
  - x arrives fp16; LayerNorm folds into the PE transpose: DVE centers x
    (x - mu, per-partition scalar), PE multiplies by diag(rstd) while
    transposing (fp16 matmul). No separate LN-apply pass.
  - rstd comes from a DVE-only Newton rsqrt (seed 1.5 - a/2 clamped, 4
    iters): the ACT engine never leaves the exp table set (each walrus
    table switch costs ~1.3us and Ln/Exp live in different sets here).
  - attention emission interleaves with phase-1 groups: the exp stream
    starts once groups 0-1 are projected; groups 2-3 + the cc=1 qk pieces
    ride the per-slot PE slack of later units.
  - during the ramp the (otherwise idle) ACT engine does all PSUM
    evacuations so the DVE queue never gates the PE via the dps rotation.
  - per J all dots share one kT weight load; v padded to 128 columns for
    fast weight load; softmax denominator = ones column of v (av row 64);
    unnormalized softmax is safe (|scores| <~ 10).
  - denominator broadcast via gpsimd partition_broadcast (needs a
    partition-0 staging row; it replicates the physical partition 0).
  - h3 runs i-halves sequentially: out-proj + stores of ih0 overlap the
    ih1 exp stream; the final boundary reads the PSUM accumulator
    directly to shorten the serial tail.
  - single shared PSUM pool: dps tag 2x[128,1024] + av tag 2x[128,1024]
    = 8 banks; transposes/QKV/out-proj carve regions of dps-tag tiles.
  - identity matrix, fp16 x, bf16 weights and weight layouts prepped on
    host; x t0-7 on the sync queue, t8-15 + weights batched on gpsimd.
"""

import os
import sys
from contextlib import ExitStack

import numpy as np

for _p in ("/opt/trn_rl_repo", "/root/.axon_site/_ro/trn_rl_repo"):
    if os.path.isdir(_p) and _p not in sys.path:
        sys.path.insert(0, _p)

import ml_dtypes

import concourse.bass as bass
import concourse.bacc as bacc
import concourse.tile as tile
from concourse import mybir
from concourse.bass_utils import run_bass_kernel_spmd

F32 = mybir.dt.float32
F16 = mybir.dt.float16
BF16 = mybir.dt.bfloat16
AF = mybir.ActivationFunctionType
OP = mybir.AluOpType
BFNP = ml_dtypes.bfloat16

B, N, DIM = 4, 2048, 512
HEADS, DH = 8, 64
EPS = 1e-5
NT = N // 128           # 16 n-tiles
DC = DIM // 128         # 4 d-chunks
SCALE = DH ** -0.5
NCORES = 8


def _emit(tc: tile.TileContext, ctx: ExitStack, aps: dict, affine: bool):
    nc = tc.nc

    const = ctx.enter_context(tc.tile_pool(name="const", bufs=1))
    big = ctx.enter_context(tc.tile_pool(name="big", bufs=1))

    # ---- weights / constants (gpsimd HWDGE queue; sync queue owns x) ----
    wq_sb = const.tile([128, DC, 256], BF16)
    wk_sb = const.tile([128, DC, 256], BF16)
    wv_sb = const.tile([128, DC, 256], BF16)
    wo_sb = const.tile([128, 2, 512], BF16)
    # one batched DMA per weight tensor: each gpsimd dma_start costs ~630ns
    # of engine time, so 14 separate triggers would stall the queue ~9us
    nc.gpsimd.dma_start(out=wq_sb[:, :, :],
                        in_=aps["wq"].rearrange("(dc p) c -> p dc c", p=128))
    nc.gpsimd.dma_start(out=wk_sb[:, :, :],
                        in_=aps["wk"].rearrange("(dc p) c -> p dc c", p=128))
    nc.gpsimd.dma_start(out=wv_sb[:, :, :],
                        in_=aps["wv"].rearrange("(dc p) c -> p dc c", p=128))
    nc.gpsimd.dma_start(out=wo_sb[:, :, :],
                        in_=aps["wo"].rearrange("(cc p) c -> p cc c", p=128))
    cadd_sb = None
    ones_row = None
    if affine:
        cadd_sb = const.tile([1, 768], BF16)     # beta @ w rows: q|k|v
        nc.gpsimd.dma_start(out=cadd_sb[:, :], in_=aps["cadd"][:, :])
        ones_row = const.tile([1, N], BF16)
        nc.vector.memset(ones_row, 1.0)
    pb_sb = const.tile([128, NT], F32)
    nc.gpsimd.dma_start(out=pb_sb[:, :], in_=aps["pb"].rearrange("(t p) -> p t", p=128))

    # ---- persistent activations ----
    x_sb = big.tile([128, NT, DIM], F16)         # 16 KB/part
    xT = big.tile([128, DC, N], BF16)            # 16 KB/part
    qT = big.tile([128, 2, N], BF16)             # 8 KB/part
    kT = big.tile([128, 2, N], BF16)
    v_sb = big.tile([128, NT, 4, 128], BF16)     # 16 KB/part (64 v | ones | 0pad)
    aoT = big.tile([128, 2, N], BF16)
    stats = const.tile([128, NT, 2], F32)
    xsum = const.tile([128, 8], F32)
    xsq = const.tile([128, 8], F32)
    lnv = const.tile([128, NT], F32)
    rstd = const.tile([128, NT], F32)
    negmu = const.tile([128, NT], F32)
    eps_sb = const.tile([128, 1], F32)
    nc.vector.memset(eps_sb, EPS)
    zero_sb = const.tile([128, 1], F32)
    nc.vector.memset(zero_sb, 0.0)
    ident16 = const.tile([128, 128], F16)
    nc.sync.dma_start(out=ident16[:, :], in_=aps["ident"][:, :])
    # zero v_sb on the ACT engine (idle until the first exp ~20us); the ones
    # column on DVE (tiny). Keeps both the DVE queue and gpsimd queue clear.
    # only the pad columns need zeroing (0-63 are written by the v evacs,
    # col 64 becomes the ones column right after)
    nc.scalar.memzero(v_sb[:, :, :, 64:128])
    nc.vector.memset(v_sb[:, :, :, 64:65], 1.0)

    # all x loads issued up-front, split across four HWDGE queues
    # x t0-7 on sync (feeds groups 0-1); t8-15 on gpsimd behind the weights
    for t in range(8):
        nc.sync.dma_start(out=x_sb[:, t, :], in_=aps["x"][t * 128:(t + 1) * 128, :])
    for t in range(8, NT):
        nc.gpsimd.dma_start(out=x_sb[:, t, :], in_=aps["x"][t * 128:(t + 1) * 128, :])

    # ---- pools (whole-kernel scope; PSUM: 2x2 + 2x2 = 8 banks) ----
    ps_pool = ctx.enter_context(tc.tile_pool(name="ps", bufs=2, space="PSUM"))
    av_pool = ctx.enter_context(tc.tile_pool(name="avps", bufs=2, space="PSUM"))
    epool = ctx.enter_context(tc.tile_pool(name="epool", bufs=6))
    ph1 = ctx.enter_context(tc.tile_pool(name="ph1", bufs=3))
    avsb_pool = ctx.enter_context(tc.tile_pool(name="avsb", bufs=2))
    rbc_pool = ctx.enter_context(tc.tile_pool(name="rbc", bufs=2))
    ostage = ctx.enter_context(tc.tile_pool(name="ostage", bufs=4))
    dram_pool = ctx.enter_context(tc.tile_pool(name="dramb", bufs=2, space="DRAM"))

    def dps_tile(name):
        return ps_pool.tile([128, 1024], F32, tag="dps", name=name)

    # ---------------- phase-1 building blocks ----------------
    def bn_tile(t):
        st6 = ph1.tile([128, 6], F32, tag="bnst")
        nc.vector.bn_stats(out=st6, in_=x_sb[:, t, :])
        nc.vector.bn_aggr(out=stats[:, t, :], in_=st6)

    def act_stats_tile(t):
        # ramp tiles: sum and sum-of-squares ride the (idle) ACT engine's
        # accum_out reduction, freeing the DVE queue which gates the ramp
        dum = ph1.tile([128, DIM], F16, tag="dum")
        nc.scalar.activation(out=dum, in_=x_sb[:, t, :], func=AF.Identity,
                             accum_out=xsum[:, t:t + 1])
        dum2 = ph1.tile([128, DIM], F16, tag="dum")
        nc.scalar.activation(out=dum2, in_=x_sb[:, t, :], func=AF.Square,
                             accum_out=xsq[:, t:t + 1])

    def moments_from_act(sl, w):
        # negmu = -sum/512 ; a = sumsq/512 + eps - mu^2  (tiny DVE ops)
        nc.vector.tensor_scalar(out=negmu[:, sl], in0=xsum[:, sl],
                                scalar1=-1.0 / DIM, scalar2=None, op0=OP.mult)
        msq = ph1.tile([128, w], F32, tag="nwt")
        nc.vector.tensor_tensor(out=msq, in0=negmu[:, sl], in1=negmu[:, sl], op=OP.mult)
        a = lnv[:, sl]
        nc.vector.tensor_scalar(out=a, in0=xsq[:, sl], scalar1=1.0 / DIM,
                                scalar2=EPS, op0=OP.mult, op1=OP.add)
        nc.vector.tensor_tensor(out=a, in0=a, in1=msq, op=OP.subtract)

    def rsqrt_slice(sl, w, from_act=False):
        # rstd via DVE-only Newton rsqrt: the ACT engine never leaves the exp
        # table set (a table load costs 1.3us and walrus reloads per switch).
        # seed y0 = clamp(1.5 - a/2, >=0.2): exact at a=1 (LN variance ~1)
        a = lnv[:, sl]
        if from_act:
            moments_from_act(sl, w)
        else:
            nc.vector.tensor_scalar(out=a, in0=stats[:, sl, 1],
                                    scalar1=EPS, scalar2=None, op0=OP.add)
        y = rstd[:, sl]
        nc.vector.tensor_scalar(out=y, in0=a, scalar1=-0.5, scalar2=1.5,
                                op0=OP.mult, op1=OP.add)
        nc.vector.tensor_scalar(out=y, in0=y, scalar1=0.2, scalar2=None,
                                op0=OP.max)
        for _ in range(4):
            t2 = ph1.tile([128, w], F32, tag="nwt")
            nc.vector.tensor_tensor(out=t2, in0=y, in1=y, op=OP.mult)
            nc.vector.tensor_tensor(out=t2, in0=t2, in1=a, op=OP.mult)
            nc.vector.tensor_scalar(out=t2, in0=t2, scalar1=-0.5, scalar2=1.5,
                                    op0=OP.mult, op1=OP.add)
            nc.vector.tensor_tensor(out=y, in0=y, in1=t2, op=OP.mult)
        if not from_act:
            nc.vector.tensor_scalar(out=negmu[:, sl], in0=stats[:, sl, 0],
                                    scalar1=-1.0, scalar2=None, op0=OP.mult)

    def stats_group(g):
        for t in range(4 * g, 4 * g + 4):
            bn_tile(t)
        rsqrt_slice(slice(4 * g, 4 * g + 4), 4)

    def transpose_tile(t, ramp=False):
        # xc = x - mu (DVE, per-partition scalar); xT = xc.T @ diag(rstd) on PE
        xc = ph1.tile([128, DIM], F16, tag="xc")
        nc.vector.tensor_scalar(out=xc, in0=x_sb[:, t, :],
                                scalar1=negmu[:, t:t + 1], scalar2=None, op0=OP.add)
        diag = ph1.tile([128, 128], F16, tag="diag")
        nc.vector.tensor_scalar(out=diag, in0=ident16, scalar1=rstd[:, t:t + 1],
                                scalar2=None, op0=OP.mult)
        ps = dps_tile(f"tp_{t}")
        for dc in range(DC):
            nc.tensor.matmul(ps[:, dc * 128:(dc + 1) * 128],
                             xc[:, dc * 128:(dc + 1) * 128], diag,
                             start=True, stop=True)
        # pre-stream the ACT engine is idle: let it do the evacuations so the
        # DVE queue never gates the PE through the dps rotation
        eng = nc.scalar.copy if ramp else (lambda out, in_: nc.vector.tensor_copy(out=out, in_=in_))
        eng(out=xT[:, :, t * 128:(t + 1) * 128],
            in_=ps[:, 0:512].rearrange("p (dc c) -> p dc c", dc=4))

    def qk_piece(w_sb, woff, dst, cc, p, ramp=False):
        ps = dps_tile(f"qk_{woff}_{cc}_{p}")
        if affine:
            nc.tensor.matmul(ps[:, 0:512], cadd_sb[0:1, woff + cc * 128: woff + (cc + 1) * 128],
                             ones_row[0:1, p * 512:(p + 1) * 512], start=True, stop=False)
        for dc in range(DC):
            nc.tensor.matmul(ps[:, 0:512], w_sb[:, dc, cc * 128:(cc + 1) * 128],
                             xT[:, dc, p * 512:(p + 1) * 512],
                             start=(dc == 0 and not affine), stop=(dc == DC - 1))
        if ramp:
            nc.scalar.copy(out=dst[:, cc, p * 512:(p + 1) * 512], in_=ps[:, 0:512])
        else:
            nc.vector.tensor_copy(out=dst[:, cc, p * 512:(p + 1) * 512], in_=ps[:, 0:512])

    def v_chunk(t, ramp=False):
        ps = dps_tile(f"v_{t}")
        if affine:
            nc.tensor.matmul(ps[:, 0:256], ones_row[0:1, t * 128:(t + 1) * 128],
                             cadd_sb[0:1, 512:768], start=True, stop=False)
        for dc in range(DC):
            nc.tensor.matmul(ps[:, 0:256], xT[:, dc, t * 128:(t + 1) * 128],
                             wv_sb[:, dc, :], start=(dc == 0 and not affine),
                             stop=(dc == DC - 1))
        # one strided copy: [128, 4, 64] view of v_sb (ACT pre-stream)
        if ramp:
            nc.scalar.copy(out=v_sb[:, t, :, 0:64],
                           in_=ps[:, 0:256].rearrange("p (h d) -> p h d", h=4))
        else:
            nc.vector.tensor_copy(out=v_sb[:, t, :, 0:64],
                                  in_=ps[:, 0:256].rearrange("p (h d) -> p h d", h=4))

    def phase1_group(g, ccs, defer_v=False, ramp=False):
        stats_group(g)
        for t in range(4 * g, 4 * g + 4):
            transpose_tile(t, ramp=ramp)
        for cc in ccs:
            qk_piece(wq_sb, 0, qT, cc, g, ramp=ramp)
            qk_piece(wk_sb, 256, kT, cc, g, ramp=ramp)
        if not defer_v:
            for t in range(4 * g, 4 * g + 4):
                v_chunk(t, ramp=ramp)

    # ---------------- attention building blocks ----------------
    def emit_dots_exp(h, ih, J):
        cc, off = h // 2, 64 * (h % 2)
        ps = dps_tile(f"d_{h}_{ih}_{J}")
        for p in range(2):
            nc.tensor.matmul(
                ps[:, p * 512:(p + 1) * 512],
                kT[off:off + 64, cc, J * 128:(J + 1) * 128],
                qT[off:off + 64, cc, ih * 1024 + p * 512: ih * 1024 + (p + 1) * 512],
                start=True, stop=True)
        eT = epool.tile([128, 1024], BF16, tag="e")
        nc.scalar.activation(out=eT, in_=ps, func=AF.Exp,
                             bias=pb_sb[:, J:J + 1], scale=SCALE)
        return eT

    def emit_av(h, av_ps, J, eT):
        for p in range(2):
            nc.tensor.matmul(av_ps[:, p * 512:(p + 1) * 512],
                             v_sb[:, J, h, :], eT[:, p * 512:(p + 1) * 512],
                             start=(J == 0), stop=(J == NT - 1))

    def boundary(h, ih, av_ps, use_act=False, final=False):
        # av -> aoT (normalized); denominator row 64 -> bcast -> reciprocal
        cc, off = h // 2, 64 * (h % 2)
        if final:
            # last unit: read the PSUM accumulator directly in the normalize
            # multiply (shortens the serial tail after the last exp)
            av_sb = av_ps
            den_row = rbc_pool.tile([1, 1024], F32, tag="drow")
            nc.vector.tensor_copy(out=den_row, in_=av_ps[64:65, :])
        else:
            av_sb = avsb_pool.tile([65, 1024], F32, tag="avsb")
            if use_act:
                nc.scalar.copy(out=av_sb, in_=av_ps[0:65, :])
            else:
                nc.vector.tensor_copy(out=av_sb, in_=av_ps[0:65, :])
            den_row = rbc_pool.tile([1, 1024], F32, tag="drow")
            nc.vector.tensor_copy(out=den_row, in_=av_sb[64:65, :])
        d_bc = rbc_pool.tile([64, 1024], F32, tag="dbc")
        # partition_broadcast replicates the tile's physical partition 0, so
        # the denominator must be copied to a partition-0 tile first
        nc.gpsimd.partition_broadcast(d_bc, den_row[:, :])
        r_bc = rbc_pool.tile([64, 1024], F32, tag="rbc")
        nc.vector.reciprocal_approx_fast(out=r_bc, in_=d_bc)
        nc.vector.tensor_tensor(out=aoT[off:off + 64, cc, ih * 1024:(ih + 1) * 1024],
                                in0=av_sb[0:64, :], in1=r_bc, op=OP.mult)

    def outproj_piece(p, mc, tail=False):
        po = dps_tile(f"op_{p}_{mc}")
        for ccx in range(2):
            nc.tensor.matmul(po[:, 0:512], wo_sb[:, ccx, mc * 128:(mc + 1) * 128],
                             aoT[:, ccx, p * 512:(p + 1) * 512],
                             start=(ccx == 0), stop=(ccx == 1))
        st = ostage.tile([128, 512], BF16, tag="ost")
        if tail and mc % 2 == 0:
            nc.scalar.copy(out=st, in_=po[:, 0:512])
        else:
            nc.vector.tensor_copy(out=st, in_=po[:, 0:512])
        oeng = nc.sync if mc % 2 == 0 else nc.scalar
        oeng.dma_start(out=aps["out"][mc * 128:(mc + 1) * 128, p * 512:(p + 1) * 512],
                       in_=st)

    # sequential-unit emitter with a 2-deep AV software pipeline
    class Unit:
        def __init__(self, h, ih):
            self.h, self.ih = h, ih
            self.av = av_pool.tile([128, 1024], F32, tag="av", name=f"av_{h}_{ih}")
            self.pend = []

        def run(self, Js, extra=None):
            for idx, J in enumerate(Js):
                eT = emit_dots_exp(self.h, self.ih, J)
                if len(self.pend) >= 2:
                    Jp, eTp = self.pend.pop(0)
                    emit_av(self.h, self.av, Jp, eTp)
                self.pend.append((J, eT))
                if extra is not None:
                    extra(idx)

        def finish(self, use_act=False, final=False):
            for Jp, eTp in self.pend:
                emit_av(self.h, self.av, Jp, eTp)
            self.pend = []
            boundary(self.h, self.ih, self.av, use_act=use_act, final=final)

    # ---------------- schedule ----------------
    # PE warmup: dummy matmuls as soon as the identity lands (~7us), then
    # more keyed on each landing x tile so the PE stays busy (HAM warm)
    # through the stats window until the real transposes start.
    wu = dps_tile("warmup")
    for i in range(16):
        nc.tensor.matmul(wu[:, (i % 4) * 128:(i % 4) * 128 + 128],
                         ident16, ident16, start=True, stop=True)
    phase1_group(0, ccs=[0], ramp=True)
    phase1_group(1, ccs=[0], ramp=True)
    phase1_group(1, ccs=[0], ramp=True)

    u = Unit(0, 0)
    u.run(range(0, 8))
    phase1_group(2, ccs=[0])
    qk_piece(wq_sb, 0, qT, 1, 0)
    qk_piece(wk_sb, 256, kT, 1, 0)
    u.run(range(8, 12))
    phase1_group(3, ccs=[0], defer_v=True)
    qk_piece(wq_sb, 0, qT, 1, 1)

    def u3_extra(idx):
        if idx < 4:
            v_chunk(12 + idx)

    u.run(range(12, 16), extra=u3_extra)
    u.finish()

    u = Unit(0, 1)

    def u01_extra(idx):
        if idx == 4:
            qk_piece(wk_sb, 256, kT, 1, 1)
        elif idx == 10:
            qk_piece(wq_sb, 0, qT, 1, 2)

    u.run(range(0, 16), extra=u01_extra)
    u.finish()

    # remaining cc=1 qk pieces ride the small per-J slack in h1
    h1_pieces = [(wk_sb, 256, kT, 2), (wq_sb, 0, qT, 3), (wk_sb, 256, kT, 3)]

    def h1_extra(idx):
        if idx % 5 == 2 and idx // 5 < len(h1_pieces):
            w, woff, dst, g = h1_pieces[idx // 5]
            qk_piece(w, woff, dst, 1, g)

    # h1, h2: interleaved i-halves (4 dots share one kT load per J)
    for h in (1, 2):
        u0, u1 = Unit(h, 0), Unit(h, 1)
        pend = []
        for J in range(NT):
            e0 = emit_dots_exp(h, 0, J)
            e1 = emit_dots_exp(h, 1, J)
            if len(pend) >= 2:
                Jp, p0, p1 = pend.pop(0)
                emit_av(h, u0.av, Jp, p0)
                emit_av(h, u1.av, Jp, p1)
            pend.append((J, e0, e1))
            if h == 1:
                h1_extra(J)
        for Jp, p0, p1 in pend:
            emit_av(h, u0.av, Jp, p0)
            emit_av(h, u1.av, Jp, p1)
        boundary(h, 0, u0.av)
        boundary(h, 1, u1.av)

    # h3: sequential i-halves; out-proj(ih0) overlaps ih1's exp stream
    u = Unit(3, 0)
    u.run(range(0, 16))
    u.finish()

    u = Unit(3, 1)
    op_state = {"i": 0}

    def h3_extra(idx):
        # start emitting ih0 out-proj pieces once the normalize chain has had
        # time to land (after a few J); one piece every other J
        if idx >= 6 and idx % 2 == 0 and op_state["i"] < 8:
            i = op_state["i"]
            outproj_piece(i // 4, i % 4)
            op_state["i"] += 1

    u.run(range(0, 16), extra=h3_extra)
    u.finish(use_act=True, final=True)
    wu2 = dps_tile("warmup2")
    for i in range(24):
        nc.tensor.matmul(wu2[:, (i % 4) * 128:(i % 4) * 128 + 128],
                         ident16, ident16, start=True, stop=True)
    while op_state["i"] < 8:
        i = op_state["i"]
        outproj_piece(i // 4, i % 4)
        op_state["i"] += 1
    for i in range(8):
        outproj_piece(2 + i // 4, i % 4, tail=True)

    if "d_xT" in aps:  # debug dumps
        nc.sync.dma_start(out=aps["d_rstd"][:, :], in_=rstd[:, :])
        for dc in range(DC):
            nc.sync.dma_start(out=aps["d_xT"][:, dc, :], in_=xT[:, dc, :])
        for cc in range(2):
            nc.sync.dma_start(out=aps["d_qT"][:, cc, :], in_=qT[:, cc, :])
            nc.sync.dma_start(out=aps["d_kT"][:, cc, :], in_=kT[:, cc, :])
            nc.sync.dma_start(out=aps["d_aoT"][:, cc, :], in_=aoT[:, cc, :])
        for t in range(NT):
            nc.sync.dma_start(out=aps["d_v"][:, t, :, :], in_=v_sb[:, t, :, :])


_CACHE: dict = {}


def _build(affine: bool):
    key = ("nc", affine)
    if key in _CACHE:
        return _CACHE[key]
    nc = bacc.Bacc("TRN2", target_bir_lowering=False, debug=False,
                   num_devices=NCORES)
    aps = {
        "x": nc.dram_tensor("x", [N, DIM], F16, kind="ExternalInput").ap(),
        "pb": nc.dram_tensor("pb", [N], F32, kind="ExternalInput").ap(),
        "wq": nc.dram_tensor("wq", [DIM, 256], BF16, kind="ExternalInput").ap(),
        "wk": nc.dram_tensor("wk", [DIM, 256], BF16, kind="ExternalInput").ap(),
        "wv": nc.dram_tensor("wv", [DIM, 256], BF16, kind="ExternalInput").ap(),
        "wo": nc.dram_tensor("wo", [256, DIM], BF16, kind="ExternalInput").ap(),
        "ident": nc.dram_tensor("ident", [128, 128], F16, kind="ExternalInput").ap(),
        "out": nc.dram_tensor("out", [DIM, N], BF16, kind="ExternalOutput").ap(),
    }
    if affine:
        aps["cadd"] = nc.dram_tensor("cadd", [1, 768], BF16, kind="ExternalInput").ap()
    with tile.TileContext(nc) as tc:
        with ExitStack() as ctx:
            _emit(tc, ctx, aps, affine)
    nc.compile()
    _CACHE[key] = nc
    return nc


def _prep_in_maps(x, pose_bias, ln_gamma, ln_beta, w_qkv, w_out, beta):
    x = np.asarray(x, np.float32)
    pose = np.asarray(pose_bias, np.float32)
    gam = np.asarray(ln_gamma, np.float32)
    bet = np.asarray(ln_beta, np.float32)
    wqkv = np.asarray(w_qkv, np.float32)
    wo = np.asarray(w_out, np.float32)
    bval = float(np.asarray(beta))
    affine = not (np.all(gam == 1.0) and np.all(bet == 0.0))
    wqkv_eff = wqkv * gam[:, None] if affine else wqkv
    wq_b = wqkv_eff[:, 0:512].astype(BFNP)
    wk_b = wqkv_eff[:, 512:1024].astype(BFNP)
    wv_b = wqkv_eff[:, 1024:1536].astype(BFNP)
    if affine:
        cadd = bet @ wqkv  # [1536] f32
    in_maps = []
    for c in range(NCORES):
        b, g = c // 2, c % 2
        sl = slice(g * 256, (g + 1) * 256)
        m = {
            "x": np.ascontiguousarray(x[b]).astype(np.float16),
            "pb": np.ascontiguousarray(bval * pose[b]),
            "wq": np.ascontiguousarray(wq_b[:, sl]),
            "wk": np.ascontiguousarray(wk_b[:, sl]),
            "wv": np.ascontiguousarray(wv_b[:, sl]),
            "wo": np.ascontiguousarray(wo[sl, :]).astype(BFNP),
            "ident": np.eye(128, dtype=np.float16),
        }
        if affine:
            m["cadd"] = np.ascontiguousarray(
                np.concatenate([cadd[0:512][sl], cadd[512:1024][sl],
                                cadd[1024:1536][sl]])[None, :].astype(BFNP))
        in_maps.append(m)
    return in_maps, affine


def _gather(results):
    outs = []
    for b in range(B):
        o = results[2 * b]["out"].astype(np.float32) + results[2 * b + 1]["out"].astype(np.float32)
        outs.append(o.T)
    return np.ascontiguousarray(np.stack(outs))


def _ensure_ntff_shim():
    import types
    if "antenv.axon_hooks" in sys.modules:
        return
    mod = types.ModuleType("antenv.axon_hooks")
    state = {"hook": None}
    mod.set_axon_ntff_profile_hook = lambda h: state.__setitem__("hook", h)
    mod.get_axon_ntff_profile_hook = lambda: state["hook"]
    sys.modules["antenv.axon_hooks"] = mod
    try:
        from trn_agent_boot.trn_boot import _ntff_profile_via_ctypes
        mod.set_axon_ntff_profile_hook(
            _ntff_profile_via_ctypes("/opt/axon/libaxon_pjrt.so"))
    except Exception:
        pass


def run(trace=False, **inputs):
    if trace:
        _ensure_ntff_shim()
    in_maps, affine = _prep_in_maps(**inputs)
    nc = _build(affine)
    res = run_bass_kernel_spmd(nc, in_maps, core_ids=list(range(NCORES)),
                               trace=trace)
    return _gather(res.results), res


def kernel(**inputs) -> np.ndarray:
    out, _ = run(trace=False, **inputs)
    return out
